# revision 2
# baseline (speedup 1.0000x reference)
"""Trainium2 Bass kernel for ConvertedLlamaAttention (LoRA q/k/v + RoPE + causal attention + out-proj).

Strategy: tensor-parallel over heads across 8 NeuronCores (4 heads/core),
single fused pass per 512-token sequence chunk:
  [QKV projections + RoPE] -> [attention for that q-chunk] -> [out-proj rows]

This version runs the projection and out-proj matmuls in fp8(e4m3) with
3-pass error compensation at DoubleRow (2x) throughput:
  W ~= Wh + Wl,  x ~= xh + xl  (both splits quantized to e4m3 on host)
  W^T x ~= Wh^T xh + Wh^T xl + Wl^T xh      (the Wl^T xl term is ~0.1% and dropped)
Three fp8 DoubleRow passes cost 0.75x of one bf16 pass, at near-bf16 accuracy.
Scales: weights carry x64 (pow2; dequant folded into the RoPE cos/sin table,
the V psum->SBUF copy, and the final out copy), attn carries x8 via the
reciprocal-broadcast ones row. Attention itself (scores/exp/AV) stays bf16.
LoRA (incl. the half/interleave permutation) is folded into the weights on
the host; per-core partial outputs are summed on the host (row-parallel Wo).
"""
import sys

for _p in ("/opt/trn_rl_repo", "/root/.axon_site/_ro/trn_rl_repo"):
    if _p not in sys.path:
        sys.path.insert(0, _p)

import numpy as np
import ml_dtypes

import concourse.bass as bass  # noqa: F401  (registers types)
import concourse.mybir as mybir
import concourse.tile as tile
from concourse import bacc, bass_utils

F32 = mybir.dt.float32
F32R = mybir.dt.float32r
BF16 = mybir.dt.bfloat16
FP8 = mybir.dt.float8e4
Exp = mybir.ActivationFunctionType.Exp
DR = mybir.MatmulPerfMode.DoubleRow

H = 4096          # hidden
S = 2048          # sequence
P = 128           # partitions
HD = 128          # head dim
NCORES = 8
HPC = 4           # heads per core
CW = HPC * HD     # per-core width of q/k/v/attn dims = 512
NCH = 4           # seq chunks of 512
KCH = H // P      # 32 hidden chunks
LORA_SCALING = 2.0
EXP_SCALE = float(1.0 / np.sqrt(HD))
WS = 64.0         # weight fp8 scale (pow2)
AS = 8.0          # attn fp8 scale (pow2)

_CACHE = {}


def _build():
    nc = bacc.Bacc("TRN2", target_bir_lowering=False, debug=False, num_devices=NCORES)

    xh_d = nc.declare_dram_parameter("xh", [H, S], FP8, isOutput=False)
    xl_d = nc.declare_dram_parameter("xl", [H, S], FP8, isOutput=False)
    wqh_d = nc.declare_dram_parameter("wqh", [H, CW], FP8, isOutput=False)
    wql_d = nc.declare_dram_parameter("wql", [H, CW], FP8, isOutput=False)
    wkh_d = nc.declare_dram_parameter("wkh", [H, CW], FP8, isOutput=False)
    wkl_d = nc.declare_dram_parameter("wkl", [H, CW], FP8, isOutput=False)
    wvh_d = nc.declare_dram_parameter("wvh", [H, CW], FP8, isOutput=False)
    wvl_d = nc.declare_dram_parameter("wvl", [H, CW], FP8, isOutput=False)
    woth_d = nc.declare_dram_parameter("woth", [CW, H], FP8, isOutput=False)
    wotl_d = nc.declare_dram_parameter("wotl", [CW, H], FP8, isOutput=False)
    cs_d = nc.declare_dram_parameter("cs", [P, S], F32, isOutput=False)
    tri_d = nc.declare_dram_parameter("tri", [P, P], BF16, isOutput=False)
    onesb_d = nc.declare_dram_parameter("onesb", [P, 1], BF16, isOutput=False)
    onesr_d = nc.declare_dram_parameter("onesr", [1, P], F32R, isOutput=False)
    out_d = nc.declare_dram_parameter("out", [S, H], BF16, isOutput=True)

    xh3 = xh_d.rearrange("(ko p) s -> p ko s", p=P)      # (128, 32, 2048)
    xl3 = xl_d.rearrange("(ko p) s -> p ko s", p=P)
    wqh3 = wqh_d.rearrange("(ko p) m -> p ko m", p=P)    # (128, 32, 512)
    wql3 = wql_d.rearrange("(ko p) m -> p ko m", p=P)
    wkh3 = wkh_d.rearrange("(ko p) m -> p ko m", p=P)
    wkl3 = wkl_d.rearrange("(ko p) m -> p ko m", p=P)
    wvh3 = wvh_d.rearrange("(ko p) m -> p ko m", p=P)
    wvl3 = wvl_d.rearrange("(ko p) m -> p ko m", p=P)
    woth3 = woth_d.rearrange("(h p) n -> p h n", p=P)    # (128, 4, 4096)
    wotl3 = wotl_d.rearrange("(h p) n -> p h n", p=P)
    out3 = out_d.rearrange("(a qs p) n -> p (a qs) n", p=P, qs=4)  # (128, 16, 4096)

    with tile.TileContext(nc) as tc:
        with tc.tile_pool(name="persist", bufs=1) as pp, \
             tc.tile_pool(name="xtp", bufs=18) as xtp, \
             tc.tile_pool(name="wqkp", bufs=10) as wqkp, \
             tc.tile_pool(name="wvp", bufs=17) as wvp, \
             tc.tile_pool(name="ropep", bufs=1) as ropep, \
             tc.tile_pool(name="probsp", bufs=4) as probsp, \
             tc.tile_pool(name="sqp", bufs=4) as sqp, \
             tc.tile_pool(name="avsp", bufs=4) as avsp, \
             tc.tile_pool(name="avqp", bufs=3) as avqp, \
             tc.tile_pool(name="osbp", bufs=3) as osbp, \
             tc.tile_pool(name="recp", bufs=1) as recp, \
             tc.tile_pool(name="avtp", bufs=2) as avtp, \
             tc.tile_pool(name="pjps", bufs=4, space="PSUM") as pjps, \
             tc.tile_pool(name="bigps", bufs=2, space="PSUM") as bigps:

            qt = [pp.tile([P, S], BF16, tag=f"qt{h}", name=f"qt{h}") for h in range(HPC)]
            kt = [pp.tile([P, S], BF16, tag=f"kt{h}", name=f"kt{h}") for h in range(HPC)]
            v_sb = pp.tile([P, S // P, CW], BF16, tag="v")   # (128, 16, 512)
            cs_sb = pp.tile([P, S], F32, tag="cs")
            tri_sb = pp.tile([P, P], BF16, tag="tri")
            onesb_sb = pp.tile([P, 1], BF16, tag="onesb")
            onesr_sb = pp.tile([1, P], F32R, tag="onesr")
            woth_sb = pp.tile([P, HPC, H], FP8, tag="woth")
            wotl_sb = pp.tile([P, HPC, H], FP8, tag="wotl")

            avsh_tiles = [None, None]  # [hp] -> fp8 [P, 2, CW] high part
            avsl_tiles = [None, None]  # [hp] -> fp8 [P, 2, CW] residual
            pend = {"n": []}           # deferred normalization queue

            def emit_norm():
                if not pend["n"]:
                    return
                qcx, h, avt_sb, sums_ps = pend["n"].pop(0)
                recf = recp.tile([1, 512], F32, tag="recf", name=f"recf{qcx}_{h}")
                nc.vector.reciprocal_approx_fast(out=recf[:], in_=sums_ps[0:1, :])
                recr = recp.tile([1, 512], F32R, tag="recr", name=f"recr{qcx}_{h}")
                nc.vector.tensor_copy(recr[:], recf[:])
                rb_ps = bigps.tile([P, 2, CW], F32, tag="big", name=f"rb{qcx}_{h}")
                # onesr holds AS (=8.0): rb = AS / denom, so avs is 8x attn.
                nc.tensor.matmul(rb_ps[:, 0, :], lhsT=onesr_sb[0:1, :], rhs=recr[:],
                                 start=True, stop=True)
                hp, j = divmod(h, 2)
                if j == 0:
                    avsh_tiles[hp] = avqp.tile([P, 2, CW], FP8, tag="avsh",
                                               name=f"avsh{qcx}_{hp}")
                    avsl_tiles[hp] = avqp.tile([P, 2, CW], FP8, tag="avsl",
                                               name=f"avsl{qcx}_{hp}")
                avsb = avsp.tile([P, CW], BF16, tag="avsb", name=f"avsb{qcx}_{h}")
                nc.vector.tensor_mul(avsb[:], avt_sb[:], rb_ps[:, 0, :])
                nc.scalar.copy(avsh_tiles[hp][:, j, :], avsb[:])
                nc.vector.tensor_sub(avsl_tiles[hp][:, j, :], avsb[:],
                                     avsh_tiles[hp][:, j, :])

            def rope(qp, dest, ssl):
                t1 = ropep.tile([P, 512], F32, tag="r1", name="r1")
                t2 = ropep.tile([P, 512], F32, tag="r2", name="r2")
                nc.vector.tensor_mul(t1[0:64], qp[0:64], cs_sb[0:64, ssl])
                nc.vector.tensor_mul(t2[0:64], qp[64:128], cs_sb[64:128, ssl])
                nc.vector.tensor_sub(dest[0:64], t1[0:64], t2[0:64])
                nc.vector.tensor_mul(t1[64:128], qp[0:64], cs_sb[64:128, ssl])
                nc.vector.tensor_mul(t2[64:128], qp[64:128], cs_sb[0:64, ssl])
                nc.vector.tensor_add(dest[64:128], t1[64:128], t2[64:128])

            for ncx in range(NCH):
                ssl = slice(ncx * 512, (ncx + 1) * 512)

                # ---- stream DMAs for this chunk ----
                xh_ts, xl_ts = [], []
                for x3, lst, xtag in ((xh3, xh_ts, "xh"), (xl3, xl_ts, "xl")):
                    for b in range(8):
                        t = xtp.tile([P, 4, 512], FP8, tag="xt", name=f"{xtag}{ncx}_{b}")
                        nc.gpsimd.dma_start(t[:], x3[:, 4 * b:4 * b + 4, ssl])
                        lst.append(t)
                if ncx == 0:
                    # one-time constants; emitted after the first x tiles so
                    # the first matmuls aren't delayed.
                    nc.gpsimd.dma_start(cs_sb[:], cs_d[:])
                    nc.gpsimd.dma_start(tri_sb[:], tri_d[:])
                    nc.sync.dma_start(onesb_sb[:], onesb_d[:])
                    nc.sync.dma_start(onesr_sb[0:1, :], onesr_d[0:1, :])

                # weight streams over three DMA queues:
                #   sync:   wq (h then l per half) + wk half0 (h, l)
                #   scalar: wk half1 (h, l) + wv (h tiles, then l tiles)
                #   gpsimd: x + consts + outputs.
                wqh_t, wql_t, wkh_t, wkl_t = [], [], [], []
                for w3h, w3l, lsth, lstl, wtag in (
                        (wqh3, wql3, wqh_t, wql_t, "wq"),
                        (wkh3, wkl3, wkh_t, wkl_t, "wk")):
                    for half in (0, 1):
                        eng = nc.sync if (wtag == "wq" or half == 0) else nc.scalar
                        th, tl = [], []
                        for w3x, lx, sub in ((w3h, th, "h"), (w3l, tl, "l")):
                            for kb in range(8):
                                t = wqkp.tile([P, 4, 256], FP8, tag="w",
                                              name=f"{wtag}{sub}{ncx}_{half}_{kb}")
                                eng.dma_start(
                                    t[:], w3x[:, 4 * kb:4 * kb + 4,
                                              256 * half:256 * half + 256])
                                lx.append(t)
                        lsth.append(th)
                        lstl.append(tl)
                wvh_t, wvl_t = [], []
                for w3x, lx, sub in ((wvh3, wvh_t, "h"), (wvl3, wvl_t, "l")):
                    for kb in range(8):
                        t = wvp.tile([P, 4, 512], FP8, tag="wv",
                                     name=f"wv{sub}{ncx}_{kb}")
                        nc.scalar.dma_start(t[:], w3x[:, 4 * kb:4 * kb + 4, :])
                        lx.append(t)
                if ncx == 0:
                    for hh in range(HPC):
                        nc.sync.dma_start(woth_sb[:, hh, :], woth3[:, hh, :])
                        nc.sync.dma_start(wotl_sb[:, hh, :], wotl3[:, hh, :])

                # ---- projection waves: Q01 Q23 K01 K23 V(t-major) ----
                # 3-pass fp8 DoubleRow: (Wh,xh), (Wh,xl), (Wl,xh); the k dim
                # is consumed in pairs of 128-tiles (256 per instruction).
                for dst, wh_t, wl_t in ((qt, wqh_t, wql_t), (kt, wkh_t, wkl_t)):
                    for half in (0, 1):
                        ps0 = pjps.tile([P, CW], F32, tag="pj", name=f"p{ncx}_{half}0")
                        ps1 = pjps.tile([P, CW], F32, tag="pj", name=f"p{ncx}_{half}1")
                        passes = ((wh_t[half], xh_ts), (wh_t[half], xl_ts),
                                  (wl_t[half], xh_ts))
                        for px, (wlist, xlist) in enumerate(passes):
                            for pi in range(16):
                                kb, kj = divmod(2 * pi, 4)
                                wt = wlist[kb]
                                rhs = xlist[kb][:, kj:kj + 2, :]
                                st = (px == 0 and pi == 0)
                                sp = (px == 2 and pi == 15)
                                nc.tensor.matmul(ps0[:], lhsT=wt[:, kj:kj + 2, 0:128],
                                                 rhs=rhs, start=st, stop=sp,
                                                 perf_mode=DR)
                                nc.tensor.matmul(ps1[:], lhsT=wt[:, kj:kj + 2, 128:256],
                                                 rhs=rhs, start=st, stop=sp,
                                                 perf_mode=DR)
                        rope(ps0, dst[2 * half][:, ssl], ssl)
                        rope(ps1, dst[2 * half + 1][:, ssl], ssl)

                for t4 in range(4):
                    psv = pjps.tile([P, CW], F32, tag="pj", name=f"pv{ncx}_{t4}")
                    vpasses = ((xh_ts, wvh_t), (xh_ts, wvl_t), (xl_ts, wvh_t))
                    for px, (xlist, wlist) in enumerate(vpasses):
                        for pi in range(16):
                            kb, kj = divmod(2 * pi, 4)
                            nc.tensor.matmul(
                                psv[:],
                                lhsT=xlist[kb][:, kj:kj + 2, 128 * t4:128 * t4 + 128],
                                rhs=wlist[kb][:, kj:kj + 2, :],
                                start=(px == 0 and pi == 0),
                                stop=(px == 2 and pi == 15),
                                perf_mode=DR)
                    nc.scalar.mul(v_sb[:, 4 * ncx + t4, :], psv[:], 1.0 / WS)

                # ---- attention for q-chunk qc = ncx ----
                # two heads interleaved: while one head's exp runs on ACT,
                # the PE issues the sibling head's scores/AV, so the
                # activation latency never gates the Tensor engine.
                qc = ncx
                nkt = 4 * (qc + 1)
                for hp in range(2):
                    heads = (2 * hp, 2 * hp + 1)
                    st_ = {}
                    for h in heads:
                        st_[h] = {
                            "avt": pjps.tile([P, CW], F32, tag="pj", name=f"avt{qc}_{h}"),
                            "sums": pjps.tile([P, CW], F32, tag="pj", name=f"sums{qc}_{h}"),
                            "qd_i": 0, "pp": None, "dq": None, "pav": None,
                        }

                    def emit_av(h, av):
                        for u, kti, off, probs2 in av:
                            nc.tensor.matmul(
                                st_[h]["avt"][:, off:512],
                                lhsT=v_sb[:, kti, h * HD:(h + 1) * HD],
                                rhs=probs2[:, u, off:512],
                                start=(kti == 0), stop=(kti == nkt - 1),
                                skip_group_check=True)

                    def quad_book(h, kA, kB, offA, offB, probs2):
                        stt = st_[h]
                        if kA >= 4 * qc:
                            if offA == 0:
                                stt["dq"] = sqp.tile([P, CW], BF16, tag="sq",
                                                     name=f"dq{qc}_{h}")
                                nc.vector.tensor_copy(stt["dq"][:], probs2[:, 0, :])
                            else:
                                nc.vector.tensor_add(stt["dq"][:, offA:512],
                                                     stt["dq"][:, offA:512],
                                                     probs2[:, 0, offA:512])
                            nc.vector.tensor_add(stt["dq"][:, offB:512],
                                                 stt["dq"][:, offB:512],
                                                 probs2[:, 1, offB:512])
                            if kB == nkt - 1:
                                nc.tensor.matmul(stt["sums"][0:1, :],
                                                 lhsT=onesb_sb[:, 0:1], rhs=stt["dq"][:],
                                                 start=(stt["qd_i"] == 0), stop=True)
                                stt["qd_i"] += 1
                        else:
                            psum = sqp.tile([P, CW], BF16, tag="sq",
                                            name=f"sq{qc}_{h}_{kA}")
                            nc.vector.tensor_add(psum[:], probs2[:, 0, :], probs2[:, 1, :])
                            if stt["pp"] is None:
                                stt["pp"] = psum
                            else:
                                qd = sqp.tile([P, CW], BF16, tag="sq",
                                              name=f"qd{qc}_{h}_{kA}")
                                nc.vector.tensor_add(qd[:], stt["pp"][:], psum[:])
                                stt["pp"] = None
                                nc.tensor.matmul(stt["sums"][0:1, :],
                                                 lhsT=onesb_sb[:, 0:1], rhs=qd[:],
                                                 start=(stt["qd_i"] == 0), stop=False)
                                stt["qd_i"] += 1

                    for pi in range(nkt // 2):
                        kA, kB = 2 * pi, 2 * pi + 1
                        offA = max(0, (kA - 4 * qc) * 128)
                        offB = max(0, (kB - 4 * qc) * 128)
                        for hx, h in enumerate(heads):
                            st2 = bigps.tile([P, 2, CW], F32, tag="big",
                                             name=f"st{qc}_{h}_{pi}")
                            nc.tensor.matmul(st2[:, 0, offA:512],
                                             lhsT=kt[h][:, kA * 128:(kA + 1) * 128],
                                             rhs=qt[h][:, qc * 512 + offA:(qc + 1) * 512],
                                             start=True, stop=True)
                            nc.tensor.matmul(st2[:, 1, offB:512],
                                             lhsT=kt[h][:, kB * 128:(kB + 1) * 128],
                                             rhs=qt[h][:, qc * 512 + offB:(qc + 1) * 512],
                                             start=True, stop=True)
                            if qc == 0 and hp == 0 and pi == 0 and offB > 0:
                                # first-ever touches of this PSUM ring: zero
                                # the window gap before the full-tile exp.
                                nc.vector.memset(st2[:, 1, 0:offB], 0.0)
                            probs2 = probsp.tile([P, 2, CW], BF16, tag="probs",
                                                 name=f"pr{qc}_{h}_{pi}")
                            nc.scalar.activation(probs2[:], st2[:], Exp, scale=EXP_SCALE)
                            for u, kti in ((0, kA), (1, kB)):
                                if kti >= 4 * qc:
                                    d = kti - 4 * qc
                                    nc.vector.tensor_mul(
                                        probs2[:, u, d * 128:(d + 1) * 128],
                                        probs2[:, u, d * 128:(d + 1) * 128], tri_sb[:])
                            if pi >= 1 and hx == 0:
                                emit_norm()
                            if st_[h]["pav"] is not None:
                                emit_av(h, st_[h]["pav"])
                            st_[h]["pav"] = ((0, kA, offA, probs2), (1, kB, offB, probs2))
                            quad_book(h, kA, kB, offA, offB, probs2)

                    for h in heads:
                        emit_av(h, st_[h]["pav"])
                        avt_sb = avtp.tile([P, CW], BF16, tag="avt", name=f"avtsb{qc}_{h}")
                        nc.scalar.copy(avt_sb[:], st_[h]["avt"][:])
                        pend["n"].append((qc, h, avt_sb, st_[h]["sums"]))

                # ---- out-proj for this q-chunk ----
                # fp8 DoubleRow over head-pairs, 3 passes; each lhsT slice
                # feeds the 2 column psum slices back-to-back.
                while pend["n"]:
                    emit_norm()
                for hcp in range(4):
                    for qs in range(4):
                        o2 = bigps.tile([P, 2, CW], F32, tag="big",
                                        name=f"o{qc}_{hcp}_{qs}")
                        qsl = slice(qs * 128, (qs + 1) * 128)
                        seq = []
                        for hp2 in range(2):
                            seq.append((avsh_tiles[hp2], woth_sb, hp2))
                            seq.append((avsh_tiles[hp2], wotl_sb, hp2))
                            seq.append((avsl_tiles[hp2], woth_sb, hp2))
                        for si, (a, w, hp2) in enumerate(seq):
                            for j in (0, 1):
                                hc = 2 * hcp + j
                                nc.tensor.matmul(
                                    o2[:, j, :], lhsT=a[:, :, qsl],
                                    rhs=w[:, 2 * hp2:2 * hp2 + 2,
                                          hc * 512:(hc + 1) * 512],
                                    start=(si == 0), stop=(si == len(seq) - 1),
                                    perf_mode=DR)
                        osb = osbp.tile([P, 2, CW], BF16, tag="osb",
                                        name=f"osb{qc}_{hcp}_{qs}")
                        if qs % 2 == 0 or (qc == NCH - 1 and hcp == 3):
                            nc.scalar.mul(osb[:], o2[:], 1.0 / (WS * AS))
                        else:
                            nc.vector.tensor_scalar_mul(osb[:], o2[:], 1.0 / (WS * AS))
                        nc.gpsimd.dma_start(
                            out3[:, qc * 4 + qs, hcp * 1024:(hcp + 1) * 1024], osb[:])

    nc.compile()
    return nc


def _fold(W, A, B):
    """Fold LoRA + its half/interleave permutation into the base weight."""
    BA = (B.astype(np.float64) @ A.astype(np.float64)) * LORA_SCALING
    j = np.arange(H)
    g = np.where(j < H // 2, 2 * j, 2 * (j - H // 2) + 1)
    return (W.astype(np.float64) + BA[g, :]).astype(np.float32)


def _host_consts():
    inv_freq = (1.0 / (10000.0 ** (np.arange(0, HD, 2, dtype=np.float32) / HD))).astype(np.float32)
    freqs = np.arange(S, dtype=np.float32)[:, None] * inv_freq[None, :]   # (S, 64)
    cs = np.concatenate([np.cos(freqs).T, np.sin(freqs).T], axis=0).astype(np.float32)  # (128, S)
    cs *= np.float32(1.0 / WS)  # dequant of the x64 weight scale on q/k
    tri = (np.arange(P)[:, None] <= np.arange(P)[None, :]).astype(ml_dtypes.bfloat16)
    onesb = np.ones((P, 1), dtype=ml_dtypes.bfloat16)
    onesr = np.full((1, P), AS, dtype=np.float32)
    return cs, tri, onesb, onesr


def _split8(a):
    """Split fp32 array into (high, low) e4m3 parts: a ~= high + low."""
    E4 = ml_dtypes.float8_e4m3
    hi = a.astype(E4)
    lo = (a - hi.astype(np.float32)).astype(E4)
    return np.ascontiguousarray(hi), np.ascontiguousarray(lo)


def kernel(hidden_states, Wq, Wk, Wv, Wo, Aq, Bq, Ak, Bk, Av, Bv):
    if "nc" not in _CACHE:
        _CACHE["nc"] = _build()
    nc = _CACHE["nc"]

    x = np.ascontiguousarray(np.asarray(hidden_states, dtype=np.float32)[0])  # (S, H)
    xt = np.ascontiguousarray(x.T)
    xh, xl = _split8(xt)

    Wq_eff = _fold(np.asarray(Wq), np.asarray(Aq), np.asarray(Bq))
    Wk_eff = _fold(np.asarray(Wk), np.asarray(Ak), np.asarray(Bk))
    Wv_eff = _fold(np.asarray(Wv), np.asarray(Av), np.asarray(Bv))
    Wo_np = np.asarray(Wo, dtype=np.float32)

    cs, tri, onesb, onesr = _host_consts()

    in_maps = []
    for c in range(NCORES):
        cols = slice(CW * c, CW * (c + 1))
        wqh, wql = _split8(np.ascontiguousarray(Wq_eff[cols].T) * np.float32(WS))
        wkh, wkl = _split8(np.ascontiguousarray(Wk_eff[cols].T) * np.float32(WS))
        wvh, wvl = _split8(np.ascontiguousarray(Wv_eff[cols].T) * np.float32(WS))
        woth, wotl = _split8(np.ascontiguousarray(Wo_np[:, cols].T) * np.float32(WS))
        in_maps.append({
            "xh": xh, "xl": xl,
            "wqh": wqh, "wql": wql,
            "wkh": wkh, "wkl": wkl,
            "wvh": wvh, "wvl": wvl,
            "woth": woth, "wotl": wotl,
            "cs": cs,
            "tri": tri,
            "onesb": onesb,
            "onesr": onesr,
        })
    _CACHE["in_maps"] = in_maps

    res = bass_utils.run_bass_kernel_spmd(nc, in_maps, core_ids=list(range(NCORES)))
    acc = np.zeros((S, H), dtype=np.float32)
    for c in range(NCORES):
        acc += res.results[c]["out"].astype(np.float32)
    return acc[None]


# revision 7
# speedup vs baseline: 1.3441x; 1.3441x over previous
"""Trainium2 Bass kernel for ConvertedLlamaAttention (LoRA q/k/v + RoPE + causal attention + out-proj).

Strategy: tensor-parallel over heads across 8 NeuronCores (4 heads/core),
single fused pass per 512-token sequence chunk:
  [QKV projections + RoPE] -> [attention for that q-chunk] -> [out-proj rows]
so the Tensor engine never hits a phase boundary, DMA prefetch stays ahead,
and the PE p-state stays high.

Differences vs the previous (3-phase) version:
  - everything on SBUF is bf16 (qt/kt/v/probs/avs/wot/out); PSUM stays fp32.
  - causal diagonal tiles use trimmed moving windows (no wasted columns) and
    one shared 128x128 triangle mask.
  - softmax denominators: probs pairs/quads are pre-reduced on the Vector
    engine, then a single ones-matmul per quad -> 4x less PE time on sums.
  - normalization (recip -> broadcast -> scale) is software-pipelined one
    head behind, so the PE never waits on the DVE chain.
  - out-proj is emitted per q-chunk; partial outputs stream out as bf16 and
    the host does the final fp32 reduction across cores.
LoRA (incl. the half/interleave permutation) is folded into the weights on
the host; per-core partial outputs are summed on the host (row-parallel Wo).
"""
import sys

for _p in ("/opt/trn_rl_repo", "/root/.axon_site/_ro/trn_rl_repo"):
    if _p not in sys.path:
        sys.path.insert(0, _p)

import numpy as np
import ml_dtypes

import concourse.bass as bass  # noqa: F401  (registers types)
import concourse.mybir as mybir
import concourse.tile as tile
from concourse import bacc, bass_utils

F32 = mybir.dt.float32
F32R = mybir.dt.float32r
BF16 = mybir.dt.bfloat16
Exp = mybir.ActivationFunctionType.Exp

H = 4096          # hidden
S = 2048          # sequence
P = 128           # partitions
HD = 128          # head dim
NCORES = 8
HPC = 4           # heads per core
CW = HPC * HD     # per-core width of q/k/v/attn dims = 512
NCH = 4           # seq chunks of 512
KCH = H // P      # 32 hidden chunks
LORA_SCALING = 2.0
EXP_SCALE = float(1.0 / np.sqrt(HD))

_CACHE = {}


def _build():
    nc = bacc.Bacc("TRN2", target_bir_lowering=False, debug=False, num_devices=NCORES)

    xt_d = nc.declare_dram_parameter("xt", [H, S], BF16, isOutput=False)
    wq_d = nc.declare_dram_parameter("wq", [H, CW], BF16, isOutput=False)
    wk_d = nc.declare_dram_parameter("wk", [H, CW], BF16, isOutput=False)
    wv_d = nc.declare_dram_parameter("wv", [H, CW], BF16, isOutput=False)
    wot_d = nc.declare_dram_parameter("wot", [CW, H], BF16, isOutput=False)
    cs_d = nc.declare_dram_parameter("cs", [P, S], F32, isOutput=False)
    tri_d = nc.declare_dram_parameter("tri", [P, P], BF16, isOutput=False)
    onesb_d = nc.declare_dram_parameter("onesb", [P, 1], BF16, isOutput=False)
    onesr_d = nc.declare_dram_parameter("onesr", [1, P], F32R, isOutput=False)
    out_d = nc.declare_dram_parameter("out", [S, H], BF16, isOutput=True)

    xt3 = xt_d.rearrange("(ko p) s -> p ko s", p=P)      # (128, 32, 2048)
    wq3 = wq_d.rearrange("(ko p) m -> p ko m", p=P)      # (128, 32, 512)
    wk3 = wk_d.rearrange("(ko p) m -> p ko m", p=P)
    wv3 = wv_d.rearrange("(ko p) m -> p ko m", p=P)
    wot3 = wot_d.rearrange("(h p) n -> p h n", p=P)      # (128, 4, 4096)
    out3 = out_d.rearrange("(a qs p) n -> p (a qs) n", p=P, qs=4)  # (128, 16, 4096)

    with tile.TileContext(nc) as tc:
        with tc.tile_pool(name="persist", bufs=1) as pp, \
             tc.tile_pool(name="xtp", bufs=9) as xtp, \
             tc.tile_pool(name="wqkp", bufs=8) as wqkp, \
             tc.tile_pool(name="wvp", bufs=8) as wvp, \
             tc.tile_pool(name="ropep", bufs=1) as ropep, \
             tc.tile_pool(name="probsp", bufs=4) as probsp, \
             tc.tile_pool(name="sqp", bufs=4) as sqp, \
             tc.tile_pool(name="avsp", bufs=6) as avsp, \
             tc.tile_pool(name="osbp", bufs=3) as osbp, \
             tc.tile_pool(name="recp", bufs=1) as recp, \
             tc.tile_pool(name="avtp", bufs=2) as avtp, \
             tc.tile_pool(name="pjps", bufs=4, space="PSUM") as pjps, \
             tc.tile_pool(name="bigps", bufs=2, space="PSUM") as bigps:

            qt = [pp.tile([P, S], BF16, tag=f"qt{h}", name=f"qt{h}") for h in range(HPC)]
            kt = [pp.tile([P, S], BF16, tag=f"kt{h}", name=f"kt{h}") for h in range(HPC)]
            v_sb = pp.tile([P, S // P, CW], BF16, tag="v")   # (128, 16, 512)
            cs_sb = pp.tile([P, S], F32, tag="cs")
            tri_sb = pp.tile([P, P], BF16, tag="tri")
            onesb_sb = pp.tile([P, 1], BF16, tag="onesb")
            onesr_sb = pp.tile([1, P], F32R, tag="onesr")
            wot_sb = pp.tile([P, HPC, H], BF16, tag="wot")

            avs_tiles = [None] * HPC   # normalized attn (128 hd, 512 q) of current qc
            pend = {"n": []}           # deferred normalization queue

            def emit_norm():
                if not pend["n"]:
                    return
                qcx, h, avt_sb, sums_ps = pend["n"].pop(0)
                recf = recp.tile([1, 512], F32, tag="recf", name=f"recf{qcx}_{h}")
                nc.vector.reciprocal_approx_fast(out=recf[:], in_=sums_ps[0:1, :])
                recr = recp.tile([1, 512], F32R, tag="recr", name=f"recr{qcx}_{h}")
                nc.vector.tensor_copy(recr[:], recf[:])
                rb_ps = bigps.tile([P, 2, CW], F32, tag="big", name=f"rb{qcx}_{h}")
                nc.tensor.matmul(rb_ps[:, 0, :], lhsT=onesr_sb[0:1, :], rhs=recr[:],
                                 start=True, stop=True)
                avs = avsp.tile([P, CW], BF16, tag="avs", name=f"avs{qcx}_{h}")
                nc.vector.tensor_mul(avs[:], avt_sb[:], rb_ps[:, 0, :])
                avs_tiles[h] = avs

            def rope(qp, dest, ssl):
                t1 = ropep.tile([P, 512], F32, tag="r1", name="r1")
                t2 = ropep.tile([P, 512], F32, tag="r2", name="r2")
                nc.vector.tensor_mul(t1[0:64], qp[0:64], cs_sb[0:64, ssl])
                nc.vector.tensor_mul(t2[0:64], qp[64:128], cs_sb[64:128, ssl])
                nc.vector.tensor_sub(dest[0:64], t1[0:64], t2[0:64])
                nc.vector.tensor_mul(t1[64:128], qp[0:64], cs_sb[64:128, ssl])
                nc.vector.tensor_mul(t2[64:128], qp[64:128], cs_sb[0:64, ssl])
                nc.vector.tensor_add(dest[64:128], t1[64:128], t2[64:128])

            for ncx in range(NCH):
                ssl = slice(ncx * 512, (ncx + 1) * 512)

                # ---- stream DMAs for this chunk ----
                xts = []
                for b in range(8):
                    t = xtp.tile([P, 4, 512], BF16, tag="xt", name=f"xt{ncx}_{b}")
                    nc.gpsimd.dma_start(t[:], xt3[:, 4 * b:4 * b + 4, ssl])
                    xts.append(t)
                if ncx == 0:
                    # one-time constants; emitted after the first xt tiles so
                    # the first matmuls aren't delayed.
                    nc.gpsimd.dma_start(cs_sb[:], cs_d[:])
                    nc.gpsimd.dma_start(tri_sb[:], tri_d[:])

                # weight streams ride the three DMA rings (sync/scalar HWDGE
                # + gpsimd SWDGE) so the first chunk's prefetch isn't
                # serialized behind one ring (each sustains ~150GB/s):
                #   chunk 0: sync = wq; scalar = wk then wot; gpsimd = xt, wv
                #   later:   sync = wq + wk h0; scalar = wk h1 + wv + outs
                wq_t, wk_t, wv_t = [], [], []
                for w3, lst, wtag in ((wq3, wq_t, "wq"), (wk3, wk_t, "wk")):
                    for half in (0, 1):
                        if wtag == "wq":
                            eng = nc.sync
                        elif ncx == 0:
                            eng = nc.scalar
                        else:
                            eng = nc.sync if half == 0 else nc.scalar
                        for kb in range(8):
                            t = wqkp.tile([P, 4, 256], BF16, tag="w",
                                          name=f"{wtag}{ncx}_{half}_{kb}")
                            eng.dma_start(
                                t[:], w3[:, 4 * kb:4 * kb + 4, 256 * half:256 * half + 256])
                            lst.append(t)
                veng = nc.gpsimd if ncx == 0 else nc.scalar
                for kb in range(8):
                    t = wvp.tile([P, 4, 512], BF16, tag="wv", name=f"wv{ncx}_{kb}")
                    veng.dma_start(t[:], wv3[:, 4 * kb:4 * kb + 4, :])
                    wv_t.append(t)
                if ncx == 0:
                    nc.sync.dma_start(onesb_sb[:], onesb_d[:])
                    nc.sync.dma_start(onesr_sb[0:1, :], onesr_d[0:1, :])
                    for hh in range(HPC):
                        nc.scalar.dma_start(wot_sb[:, hh, :], wot3[:, hh, :])

                # ---- projection waves: Q01 Q23 K01 K23 V(t-major) ----
                for dst, w_half in ((qt, wq_t), (kt, wk_t)):
                    for half in (0, 1):
                        ps0 = pjps.tile([P, CW], F32, tag="pj", name=f"p{ncx}_{half}0")
                        ps1 = pjps.tile([P, CW], F32, tag="pj", name=f"p{ncx}_{half}1")
                        for kb in range(8):
                            wt = w_half[8 * half + kb]
                            for ki in range(4):
                                k = 4 * kb + ki
                                rhs = xts[k // 4][:, k % 4, :]
                                nc.tensor.matmul(ps0[:], lhsT=wt[:, ki, 0:128], rhs=rhs,
                                                 start=(k == 0), stop=(k == KCH - 1))
                                nc.tensor.matmul(ps1[:], lhsT=wt[:, ki, 128:256], rhs=rhs,
                                                 start=(k == 0), stop=(k == KCH - 1))
                        rope(ps0, dst[2 * half][:, ssl], ssl)
                        rope(ps1, dst[2 * half + 1][:, ssl], ssl)

                for t4 in range(4):
                    psv = pjps.tile([P, CW], F32, tag="pj", name=f"pv{ncx}_{t4}")
                    for kb in range(8):
                        wt = wv_t[kb]
                        for ki in range(4):
                            k = 4 * kb + ki
                            nc.tensor.matmul(
                                psv[:], lhsT=xts[k // 4][:, k % 4, 128 * t4:128 * t4 + 128],
                                rhs=wt[:, ki, :], start=(k == 0), stop=(k == KCH - 1))
                    nc.scalar.copy(v_sb[:, 4 * ncx + t4, :], psv[:])

                # ---- attention for q-chunk qc = ncx ----
                # two heads interleaved: while one head's exp runs on ACT,
                # the PE issues the sibling head's scores/AV, so the
                # activation latency never gates the Tensor engine.
                qc = ncx
                nkt = 4 * (qc + 1)
                for hp in range(2):
                    heads = (2 * hp, 2 * hp + 1)
                    st_ = {}
                    for h in heads:
                        st_[h] = {
                            "avt": pjps.tile([P, CW], F32, tag="pj", name=f"avt{qc}_{h}"),
                            "sums": pjps.tile([P, CW], F32, tag="pj", name=f"sums{qc}_{h}"),
                            "qd_i": 0, "pp": None, "dq": None, "pav": None,
                        }

                    def emit_av(h, av):
                        for u, kti, off, probs2 in av:
                            nc.tensor.matmul(
                                st_[h]["avt"][:, off:512],
                                lhsT=v_sb[:, kti, h * HD:(h + 1) * HD],
                                rhs=probs2[:, u, off:512],
                                start=(kti == 0), stop=(kti == nkt - 1),
                                skip_group_check=True)

                    def quad_book(h, kA, kB, offA, offB, probs2):
                        stt = st_[h]
                        if kA >= 4 * qc:
                            if offA == 0:
                                stt["dq"] = sqp.tile([P, CW], BF16, tag="sq",
                                                     name=f"dq{qc}_{h}")
                                nc.vector.tensor_copy(stt["dq"][:], probs2[:, 0, :])
                            else:
                                nc.vector.tensor_add(stt["dq"][:, offA:512],
                                                     stt["dq"][:, offA:512],
                                                     probs2[:, 0, offA:512])
                            nc.vector.tensor_add(stt["dq"][:, offB:512],
                                                 stt["dq"][:, offB:512],
                                                 probs2[:, 1, offB:512])
                            if kB == nkt - 1:
                                nc.tensor.matmul(stt["sums"][0:1, :],
                                                 lhsT=onesb_sb[:, 0:1], rhs=stt["dq"][:],
                                                 start=(stt["qd_i"] == 0), stop=True)
                                stt["qd_i"] += 1
                        else:
                            psum = sqp.tile([P, CW], BF16, tag="sq",
                                            name=f"sq{qc}_{h}_{kA}")
                            nc.vector.tensor_add(psum[:], probs2[:, 0, :], probs2[:, 1, :])
                            if stt["pp"] is None:
                                stt["pp"] = psum
                            else:
                                qd = sqp.tile([P, CW], BF16, tag="sq",
                                              name=f"qd{qc}_{h}_{kA}")
                                nc.vector.tensor_add(qd[:], stt["pp"][:], psum[:])
                                stt["pp"] = None
                                nc.tensor.matmul(stt["sums"][0:1, :],
                                                 lhsT=onesb_sb[:, 0:1], rhs=qd[:],
                                                 start=(stt["qd_i"] == 0), stop=False)
                                stt["qd_i"] += 1

                    for pi in range(nkt // 2):
                        kA, kB = 2 * pi, 2 * pi + 1
                        offA = max(0, (kA - 4 * qc) * 128)
                        offB = max(0, (kB - 4 * qc) * 128)
                        for hx, h in enumerate(heads):
                            st2 = bigps.tile([P, 2, CW], F32, tag="big",
                                             name=f"st{qc}_{h}_{pi}")
                            nc.tensor.matmul(st2[:, 0, offA:512],
                                             lhsT=kt[h][:, kA * 128:(kA + 1) * 128],
                                             rhs=qt[h][:, qc * 512 + offA:(qc + 1) * 512],
                                             start=True, stop=True)
                            nc.tensor.matmul(st2[:, 1, offB:512],
                                             lhsT=kt[h][:, kB * 128:(kB + 1) * 128],
                                             rhs=qt[h][:, qc * 512 + offB:(qc + 1) * 512],
                                             start=True, stop=True)
                            if qc == 0 and hp == 0 and pi == 0 and offB > 0:
                                # first-ever touches of this PSUM ring: zero
                                # the window gap before the full-tile exp.
                                nc.vector.memset(st2[:, 1, 0:offB], 0.0)
                            probs2 = probsp.tile([P, 2, CW], BF16, tag="probs",
                                                 name=f"pr{qc}_{h}_{pi}")
                            nc.scalar.activation(probs2[:], st2[:], Exp, scale=EXP_SCALE)
                            for u, kti in ((0, kA), (1, kB)):
                                if kti >= 4 * qc:
                                    d = kti - 4 * qc
                                    nc.vector.tensor_mul(
                                        probs2[:, u, d * 128:(d + 1) * 128],
                                        probs2[:, u, d * 128:(d + 1) * 128], tri_sb[:])
                            if pi >= 1 and hx == 0:
                                emit_norm()
                            if st_[h]["pav"] is not None:
                                emit_av(h, st_[h]["pav"])
                            st_[h]["pav"] = ((0, kA, offA, probs2), (1, kB, offB, probs2))
                            quad_book(h, kA, kB, offA, offB, probs2)

                    for h in heads:
                        emit_av(h, st_[h]["pav"])
                        avt_sb = avtp.tile([P, CW], BF16, tag="avt", name=f"avtsb{qc}_{h}")
                        nc.scalar.copy(avt_sb[:], st_[h]["avt"][:])
                        pend["n"].append((qc, h, avt_sb, st_[h]["sums"]))

                # ---- out-proj for this q-chunk ----
                # hc-pair per PSUM tile with h-inner over both columns: each
                # avs lhsT slice feeds 2 consecutive matmuls (weight-load
                # reuse), PSUM slots still double-buffer across iterations.
                while pend["n"]:
                    emit_norm()
                for hcp in range(4):
                    for qs in range(4):
                        o2 = bigps.tile([P, 2, CW], F32, tag="big",
                                        name=f"o{qc}_{hcp}_{qs}")
                        for h in range(HPC):
                            lhs = avs_tiles[h][:, qs * 128:(qs + 1) * 128]
                            for j in (0, 1):
                                hc = 2 * hcp + j
                                nc.tensor.matmul(
                                    o2[:, j, :], lhsT=lhs,
                                    rhs=wot_sb[:, h, hc * 512:(hc + 1) * 512],
                                    start=(h == 0), stop=(h == HPC - 1))
                        osb = osbp.tile([P, 2, CW], BF16, tag="osb",
                                        name=f"osb{qc}_{hcp}_{qs}")
                        if qs % 2 == 0 or (qc == NCH - 1 and hcp == 3):
                            nc.scalar.copy(osb[:], o2[:])
                        else:
                            nc.vector.tensor_copy(osb[:], o2[:])
                        # outputs ride the scalar ring: its next-chunk weights
                        # (wk half1 / wv) aren't needed until ~27us into the
                        # next chunk, so the ~13us output drain fits in the
                        # slack -- unlike sync (wq) / gpsimd (xt), whose next
                        # tiles gate the first proj matmuls.
                        nc.scalar.dma_start(
                            out3[:, qc * 4 + qs, hcp * 1024:(hcp + 1) * 1024], osb[:])

    nc.compile()
    return nc


def _fold(W, A, B):
    """Fold LoRA + its half/interleave permutation into the base weight."""
    BA = (B.astype(np.float64) @ A.astype(np.float64)) * LORA_SCALING
    j = np.arange(H)
    g = np.where(j < H // 2, 2 * j, 2 * (j - H // 2) + 1)
    return (W.astype(np.float64) + BA[g, :]).astype(np.float32)


def _host_consts():
    inv_freq = (1.0 / (10000.0 ** (np.arange(0, HD, 2, dtype=np.float32) / HD))).astype(np.float32)
    freqs = np.arange(S, dtype=np.float32)[:, None] * inv_freq[None, :]   # (S, 64)
    cs = np.concatenate([np.cos(freqs).T, np.sin(freqs).T], axis=0).astype(np.float32)  # (128, S)
    tri = (np.arange(P)[:, None] <= np.arange(P)[None, :]).astype(ml_dtypes.bfloat16)
    onesb = np.ones((P, 1), dtype=ml_dtypes.bfloat16)
    onesr = np.ones((1, P), dtype=np.float32)
    return cs, tri, onesb, onesr


def kernel(hidden_states, Wq, Wk, Wv, Wo, Aq, Bq, Ak, Bk, Av, Bv):
    if "nc" not in _CACHE:
        _CACHE["nc"] = _build()
    nc = _CACHE["nc"]

    x = np.ascontiguousarray(np.asarray(hidden_states, dtype=np.float32)[0])  # (S, H)
    xt_bf = np.ascontiguousarray(x.T).astype(ml_dtypes.bfloat16)

    Wq_eff = _fold(np.asarray(Wq), np.asarray(Aq), np.asarray(Bq))
    Wk_eff = _fold(np.asarray(Wk), np.asarray(Ak), np.asarray(Bk))
    Wv_eff = _fold(np.asarray(Wv), np.asarray(Av), np.asarray(Bv))
    Wo_np = np.asarray(Wo, dtype=np.float32)

    cs, tri, onesb, onesr = _host_consts()

    in_maps = []
    for c in range(NCORES):
        cols = slice(CW * c, CW * (c + 1))
        in_maps.append({
            "xt": xt_bf,
            "wq": np.ascontiguousarray(Wq_eff[cols].T).astype(ml_dtypes.bfloat16),
            "wk": np.ascontiguousarray(Wk_eff[cols].T).astype(ml_dtypes.bfloat16),
            "wv": np.ascontiguousarray(Wv_eff[cols].T).astype(ml_dtypes.bfloat16),
            "wot": np.ascontiguousarray(Wo_np[:, cols].T).astype(ml_dtypes.bfloat16),
            "cs": cs,
            "tri": tri,
            "onesb": onesb,
            "onesr": onesr,
        })
    _CACHE["in_maps"] = in_maps

    res = bass_utils.run_bass_kernel_spmd(nc, in_maps, core_ids=list(range(NCORES)))
    acc = np.zeros((S, H), dtype=np.float32)
    for c in range(NCORES):
        acc += res.results[c]["out"].astype(np.float32)
    return acc[None]



# revision 10
# speedup vs baseline: 1.3539x; 1.0074x over previous
"""Trainium2 Bass kernel for ConvertedLlamaAttention (LoRA q/k/v + RoPE + causal attention + out-proj).

Strategy: tensor-parallel over heads across 8 NeuronCores (4 heads/core),
single fused pass per 512-token sequence chunk:
  [QKV projections + RoPE] -> [attention for that q-chunk] -> [out-proj rows]
so the Tensor engine never hits a phase boundary, DMA prefetch stays ahead,
and the PE p-state stays high.

Differences vs the previous (3-phase) version:
  - everything on SBUF is bf16 (qt/kt/v/probs/avs/wot/out); PSUM stays fp32.
  - causal diagonal tiles use trimmed moving windows (no wasted columns) and
    one shared 128x128 triangle mask.
  - softmax denominators: probs pairs/quads are pre-reduced on the Vector
    engine, then a single ones-matmul per quad -> 4x less PE time on sums.
  - normalization (recip -> broadcast -> scale) is software-pipelined one
    head behind, so the PE never waits on the DVE chain.
  - out-proj is emitted per q-chunk; partial outputs stream out as bf16 and
    the host does the final fp32 reduction across cores.
LoRA (incl. the half/interleave permutation) is folded into the weights on
the host; per-core partial outputs are summed on the host (row-parallel Wo).
"""
import sys

for _p in ("/opt/trn_rl_repo", "/root/.axon_site/_ro/trn_rl_repo"):
    if _p not in sys.path:
        sys.path.insert(0, _p)

import numpy as np
import ml_dtypes

import concourse.bass as bass  # noqa: F401  (registers types)
import concourse.mybir as mybir
import concourse.tile as tile
from concourse import bacc, bass_utils

F32 = mybir.dt.float32
F32R = mybir.dt.float32r
BF16 = mybir.dt.bfloat16
Exp = mybir.ActivationFunctionType.Exp

H = 4096          # hidden
S = 2048          # sequence
P = 128           # partitions
HD = 128          # head dim
NCORES = 8
HPC = 4           # heads per core
CW = HPC * HD     # per-core width of q/k/v/attn dims = 512
NCH = 4           # seq chunks of 512
KCH = H // P      # 32 hidden chunks
LORA_SCALING = 2.0
EXP_SCALE = float(1.0 / np.sqrt(HD))

_CACHE = {}


def _build():
    nc = bacc.Bacc("TRN2", target_bir_lowering=False, debug=False, num_devices=NCORES)

    # Inputs are host-pretiled so every tile DMA reads a contiguous >=2KB run
    # per partition (512B runs cap a DMA ring at ~150GB/s and starve chunk 0).
    #   xt: [ncx, b, p, ki, s512]  -> tile slice [128, 4, 512], 4KB/partition
    #   wq/wk: [half, kb, p, ki, m256] -> tile [128, 4, 256], 2KB/partition
    #   wv: [kb, p, ki, m512]      -> tile [128, 4, 512], 4KB/partition
    xt_d = nc.declare_dram_parameter("xt", [H * S // 512, 512], BF16, isOutput=False)
    wq_d = nc.declare_dram_parameter("wq", [H * CW // 256, 256], BF16, isOutput=False)
    wk_d = nc.declare_dram_parameter("wk", [H * CW // 256, 256], BF16, isOutput=False)
    wv_d = nc.declare_dram_parameter("wv", [H * CW // 512, 512], BF16, isOutput=False)
    wot_d = nc.declare_dram_parameter("wot", [CW, H], BF16, isOutput=False)
    cs_d = nc.declare_dram_parameter("cs", [P, S], F32, isOutput=False)
    tri_d = nc.declare_dram_parameter("tri", [P, P], BF16, isOutput=False)
    onesb_d = nc.declare_dram_parameter("onesb", [P, 1], BF16, isOutput=False)
    onesr_d = nc.declare_dram_parameter("onesr", [1, P], F32R, isOutput=False)
    out_d = nc.declare_dram_parameter("out", [S, H], BF16, isOutput=True)

    xt5 = xt_d.rearrange("(a b p ki) s -> p a b ki s", a=NCH, b=8, ki=4)
    wq5 = wq_d.rearrange("(hf kb p ki) m -> p hf kb ki m", hf=2, kb=8, ki=4)
    wk5 = wk_d.rearrange("(hf kb p ki) m -> p hf kb ki m", hf=2, kb=8, ki=4)
    wv4 = wv_d.rearrange("(kb p ki) m -> p kb ki m", kb=8, ki=4)
    wot3 = wot_d.rearrange("(h p) n -> p h n", p=P)      # (128, 4, 4096)
    out3 = out_d.rearrange("(a qs p) n -> p (a qs) n", p=P, qs=4)  # (128, 16, 4096)

    with tile.TileContext(nc) as tc:
        with tc.tile_pool(name="persist", bufs=1) as pp, \
             tc.tile_pool(name="xtp", bufs=9) as xtp, \
             tc.tile_pool(name="wqkp", bufs=8) as wqkp, \
             tc.tile_pool(name="wvp", bufs=8) as wvp, \
             tc.tile_pool(name="ropep", bufs=1) as ropep, \
             tc.tile_pool(name="probsp", bufs=4) as probsp, \
             tc.tile_pool(name="sqp", bufs=4) as sqp, \
             tc.tile_pool(name="avsp", bufs=6) as avsp, \
             tc.tile_pool(name="osbp", bufs=3) as osbp, \
             tc.tile_pool(name="recp", bufs=1) as recp, \
             tc.tile_pool(name="avtp", bufs=2) as avtp, \
             tc.tile_pool(name="pjps", bufs=4, space="PSUM") as pjps, \
             tc.tile_pool(name="bigps", bufs=2, space="PSUM") as bigps:

            qt = [pp.tile([P, S], BF16, tag=f"qt{h}", name=f"qt{h}") for h in range(HPC)]
            kt = [pp.tile([P, S], BF16, tag=f"kt{h}", name=f"kt{h}") for h in range(HPC)]
            v_sb = pp.tile([P, S // P, CW], BF16, tag="v")   # (128, 16, 512)
            cs_sb = pp.tile([P, S], F32, tag="cs")
            tri_sb = pp.tile([P, P], BF16, tag="tri")
            onesb_sb = pp.tile([P, 1], BF16, tag="onesb")
            onesr_sb = pp.tile([1, P], F32R, tag="onesr")
            wot_sb = pp.tile([P, HPC, H], BF16, tag="wot")

            avs_tiles = [None] * HPC   # normalized attn (128 hd, 512 q) of current qc
            pend = {"n": []}           # deferred normalization queue

            def emit_norm():
                if not pend["n"]:
                    return
                qcx, h, avt_sb, sums_ps = pend["n"].pop(0)
                recf = recp.tile([1, 512], F32, tag="recf", name=f"recf{qcx}_{h}")
                nc.vector.reciprocal_approx_fast(out=recf[:], in_=sums_ps[0:1, :])
                recr = recp.tile([1, 512], F32R, tag="recr", name=f"recr{qcx}_{h}")
                nc.vector.tensor_copy(recr[:], recf[:])
                rb_ps = bigps.tile([P, 2, CW], F32, tag="big", name=f"rb{qcx}_{h}")
                nc.tensor.matmul(rb_ps[:, 0, :], lhsT=onesr_sb[0:1, :], rhs=recr[:],
                                 start=True, stop=True)
                avs = avsp.tile([P, CW], BF16, tag="avs", name=f"avs{qcx}_{h}")
                nc.vector.tensor_mul(avs[:], avt_sb[:], rb_ps[:, 0, :])
                avs_tiles[h] = avs

            def rope(qp, dest, ssl):
                t1 = ropep.tile([P, 512], F32, tag="r1", name="r1")
                t2 = ropep.tile([P, 512], F32, tag="r2", name="r2")
                nc.vector.tensor_mul(t1[0:64], qp[0:64], cs_sb[0:64, ssl])
                nc.vector.tensor_mul(t2[0:64], qp[64:128], cs_sb[64:128, ssl])
                nc.vector.tensor_sub(dest[0:64], t1[0:64], t2[0:64])
                nc.vector.tensor_mul(t1[64:128], qp[0:64], cs_sb[64:128, ssl])
                nc.vector.tensor_mul(t2[64:128], qp[64:128], cs_sb[0:64, ssl])
                nc.vector.tensor_add(dest[64:128], t1[64:128], t2[64:128])

            for ncx in range(NCH):
                ssl = slice(ncx * 512, (ncx + 1) * 512)

                # ---- stream DMAs for this chunk ----
                xts = []
                for b in range(8):
                    t = xtp.tile([P, 4, 512], BF16, tag="xt", name=f"xt{ncx}_{b}")
                    nc.gpsimd.dma_start(t[:], xt5[:, ncx, b, :, :])
                    xts.append(t)
                if ncx == 0:
                    # one-time constants; emitted after the first xt tiles so
                    # the first matmuls aren't delayed.
                    nc.gpsimd.dma_start(cs_sb[:], cs_d[:])
                    nc.gpsimd.dma_start(tri_sb[:], tri_d[:])

                # weight streams ride the two HWDGE rings (gpsimd SWDGE
                # carries xt + consts):
                #   sync = wq + wk h0;  scalar = wk h1 + wv (+wot) + outputs
                wq_t, wk_t, wv_t = [], [], []
                for w5, lst, wtag in ((wq5, wq_t, "wq"), (wk5, wk_t, "wk")):
                    for half in (0, 1):
                        eng = nc.sync if (wtag == "wq" or half == 0) else nc.scalar
                        for kb in range(8):
                            t = wqkp.tile([P, 4, 256], BF16, tag="w",
                                          name=f"{wtag}{ncx}_{half}_{kb}")
                            eng.dma_start(t[:], w5[:, half, kb, :, :])
                            lst.append(t)
                for kb in range(8):
                    t = wvp.tile([P, 4, 512], BF16, tag="wv", name=f"wv{ncx}_{kb}")
                    nc.scalar.dma_start(t[:], wv4[:, kb, :, :])
                    wv_t.append(t)
                if ncx == 0:
                    nc.sync.dma_start(onesb_sb[:], onesb_d[:])
                    nc.sync.dma_start(onesr_sb[0:1, :], onesr_d[0:1, :])
                    for hh in range(HPC):
                        nc.scalar.dma_start(wot_sb[:, hh, :], wot3[:, hh, :])

                # ---- projection waves: Q01 Q23 K01 K23 V(t-major) ----
                for dst, w_half in ((qt, wq_t), (kt, wk_t)):
                    for half in (0, 1):
                        ps0 = pjps.tile([P, CW], F32, tag="pj", name=f"p{ncx}_{half}0")
                        ps1 = pjps.tile([P, CW], F32, tag="pj", name=f"p{ncx}_{half}1")
                        for kb in range(8):
                            wt = w_half[8 * half + kb]
                            for ki in range(4):
                                k = 4 * kb + ki
                                rhs = xts[k // 4][:, k % 4, :]
                                nc.tensor.matmul(ps0[:], lhsT=wt[:, ki, 0:128], rhs=rhs,
                                                 start=(k == 0), stop=(k == KCH - 1))
                                nc.tensor.matmul(ps1[:], lhsT=wt[:, ki, 128:256], rhs=rhs,
                                                 start=(k == 0), stop=(k == KCH - 1))
                        rope(ps0, dst[2 * half][:, ssl], ssl)
                        rope(ps1, dst[2 * half + 1][:, ssl], ssl)

                for t4 in range(4):
                    psv = pjps.tile([P, CW], F32, tag="pj", name=f"pv{ncx}_{t4}")
                    for kb in range(8):
                        wt = wv_t[kb]
                        for ki in range(4):
                            k = 4 * kb + ki
                            nc.tensor.matmul(
                                psv[:], lhsT=xts[k // 4][:, k % 4, 128 * t4:128 * t4 + 128],
                                rhs=wt[:, ki, :], start=(k == 0), stop=(k == KCH - 1))
                    nc.scalar.copy(v_sb[:, 4 * ncx + t4, :], psv[:])

                # ---- attention for q-chunk qc = ncx ----
                # two heads interleaved: while one head's exp runs on ACT,
                # the PE issues the sibling head's scores/AV, so the
                # activation latency never gates the Tensor engine.
                qc = ncx
                nkt = 4 * (qc + 1)
                for hp in range(2):
                    heads = (2 * hp, 2 * hp + 1)
                    st_ = {}
                    for h in heads:
                        st_[h] = {
                            "avt": pjps.tile([P, CW], F32, tag="pj", name=f"avt{qc}_{h}"),
                            "sums": pjps.tile([P, CW], F32, tag="pj", name=f"sums{qc}_{h}"),
                            "qd_i": 0, "pp": None, "dq": None, "pav": None,
                        }

                    def emit_av(h, av):
                        for u, kti, off, probs2 in av:
                            nc.tensor.matmul(
                                st_[h]["avt"][:, off:512],
                                lhsT=v_sb[:, kti, h * HD:(h + 1) * HD],
                                rhs=probs2[:, u, off:512],
                                start=(kti == 0), stop=(kti == nkt - 1),
                                skip_group_check=True)

                    def quad_book(h, kA, kB, offA, offB, probs2):
                        stt = st_[h]
                        if kA >= 4 * qc:
                            if offA == 0:
                                stt["dq"] = sqp.tile([P, CW], BF16, tag="sq",
                                                     name=f"dq{qc}_{h}")
                                nc.vector.tensor_copy(stt["dq"][:], probs2[:, 0, :])
                            else:
                                nc.vector.tensor_add(stt["dq"][:, offA:512],
                                                     stt["dq"][:, offA:512],
                                                     probs2[:, 0, offA:512])
                            nc.vector.tensor_add(stt["dq"][:, offB:512],
                                                 stt["dq"][:, offB:512],
                                                 probs2[:, 1, offB:512])
                            if kB == nkt - 1:
                                nc.tensor.matmul(stt["sums"][0:1, :],
                                                 lhsT=onesb_sb[:, 0:1], rhs=stt["dq"][:],
                                                 start=(stt["qd_i"] == 0), stop=True)
                                stt["qd_i"] += 1
                        else:
                            psum = sqp.tile([P, CW], BF16, tag="sq",
                                            name=f"sq{qc}_{h}_{kA}")
                            nc.vector.tensor_add(psum[:], probs2[:, 0, :], probs2[:, 1, :])
                            if stt["pp"] is None:
                                stt["pp"] = psum
                            else:
                                qd = sqp.tile([P, CW], BF16, tag="sq",
                                              name=f"qd{qc}_{h}_{kA}")
                                nc.vector.tensor_add(qd[:], stt["pp"][:], psum[:])
                                stt["pp"] = None
                                nc.tensor.matmul(stt["sums"][0:1, :],
                                                 lhsT=onesb_sb[:, 0:1], rhs=qd[:],
                                                 start=(stt["qd_i"] == 0), stop=False)
                                stt["qd_i"] += 1

                    for pi in range(nkt // 2):
                        kA, kB = 2 * pi, 2 * pi + 1
                        offA = max(0, (kA - 4 * qc) * 128)
                        offB = max(0, (kB - 4 * qc) * 128)
                        for hx, h in enumerate(heads):
                            st2 = bigps.tile([P, 2, CW], F32, tag="big",
                                             name=f"st{qc}_{h}_{pi}")
                            nc.tensor.matmul(st2[:, 0, offA:512],
                                             lhsT=kt[h][:, kA * 128:(kA + 1) * 128],
                                             rhs=qt[h][:, qc * 512 + offA:(qc + 1) * 512],
                                             start=True, stop=True)
                            nc.tensor.matmul(st2[:, 1, offB:512],
                                             lhsT=kt[h][:, kB * 128:(kB + 1) * 128],
                                             rhs=qt[h][:, qc * 512 + offB:(qc + 1) * 512],
                                             start=True, stop=True)
                            if qc == 0 and hp == 0 and pi == 0 and offB > 0:
                                # first-ever touches of this PSUM ring: zero
                                # the window gap before the full-tile exp.
                                nc.vector.memset(st2[:, 1, 0:offB], 0.0)
                            probs2 = probsp.tile([P, 2, CW], BF16, tag="probs",
                                                 name=f"pr{qc}_{h}_{pi}")
                            nc.scalar.activation(probs2[:], st2[:], Exp, scale=EXP_SCALE)
                            for u, kti in ((0, kA), (1, kB)):
                                if kti >= 4 * qc:
                                    d = kti - 4 * qc
                                    nc.vector.tensor_mul(
                                        probs2[:, u, d * 128:(d + 1) * 128],
                                        probs2[:, u, d * 128:(d + 1) * 128], tri_sb[:])
                            if pi >= 1 and hx == 0:
                                emit_norm()
                            if st_[h]["pav"] is not None:
                                emit_av(h, st_[h]["pav"])
                            st_[h]["pav"] = ((0, kA, offA, probs2), (1, kB, offB, probs2))
                            quad_book(h, kA, kB, offA, offB, probs2)

                    for h in heads:
                        emit_av(h, st_[h]["pav"])
                        avt_sb = avtp.tile([P, CW], BF16, tag="avt", name=f"avtsb{qc}_{h}")
                        nc.scalar.copy(avt_sb[:], st_[h]["avt"][:])
                        pend["n"].append((qc, h, avt_sb, st_[h]["sums"]))

                # ---- out-proj for this q-chunk ----
                # hc-pair per PSUM tile with h-inner over both columns: each
                # avs lhsT slice feeds 2 consecutive matmuls (weight-load
                # reuse), PSUM slots still double-buffer across iterations.
                while pend["n"]:
                    emit_norm()
                for hcp in range(4):
                    for qs in range(4):
                        o2 = bigps.tile([P, 2, CW], F32, tag="big",
                                        name=f"o{qc}_{hcp}_{qs}")
                        for h in range(HPC):
                            lhs = avs_tiles[h][:, qs * 128:(qs + 1) * 128]
                            for j in (0, 1):
                                hc = 2 * hcp + j
                                nc.tensor.matmul(
                                    o2[:, j, :], lhsT=lhs,
                                    rhs=wot_sb[:, h, hc * 512:(hc + 1) * 512],
                                    start=(h == 0), stop=(h == HPC - 1))
                        osb = osbp.tile([P, 2, CW], BF16, tag="osb",
                                        name=f"osb{qc}_{hcp}_{qs}")
                        if qs % 2 == 0 or (qc == NCH - 1 and hcp == 3):
                            nc.scalar.copy(osb[:], o2[:])
                        else:
                            nc.vector.tensor_copy(osb[:], o2[:])
                        # outputs ride the scalar ring: its next-chunk weights
                        # (wk half1 / wv) aren't needed until ~27us into the
                        # next chunk, so the ~13us output drain fits in the
                        # slack -- unlike sync (wq) / gpsimd (xt), whose next
                        # tiles gate the first proj matmuls.
                        nc.scalar.dma_start(
                            out3[:, qc * 4 + qs, hcp * 1024:(hcp + 1) * 1024], osb[:])

    nc.compile()
    return nc


def _fold(W, A, B):
    """Fold LoRA + its half/interleave permutation into the base weight."""
    BA = (B.astype(np.float64) @ A.astype(np.float64)) * LORA_SCALING
    j = np.arange(H)
    g = np.where(j < H // 2, 2 * j, 2 * (j - H // 2) + 1)
    return (W.astype(np.float64) + BA[g, :]).astype(np.float32)


def _host_consts():
    inv_freq = (1.0 / (10000.0 ** (np.arange(0, HD, 2, dtype=np.float32) / HD))).astype(np.float32)
    freqs = np.arange(S, dtype=np.float32)[:, None] * inv_freq[None, :]   # (S, 64)
    cs = np.concatenate([np.cos(freqs).T, np.sin(freqs).T], axis=0).astype(np.float32)  # (128, S)
    tri = (np.arange(P)[:, None] <= np.arange(P)[None, :]).astype(ml_dtypes.bfloat16)
    onesb = np.ones((P, 1), dtype=ml_dtypes.bfloat16)
    onesr = np.ones((1, P), dtype=np.float32)
    return cs, tri, onesb, onesr


def _pack_x(xt_bf):
    """[H, S] -> pretiled [(a b p ki), 512]: 4KB contiguous per partition row."""
    a = xt_bf.reshape(8, 4, P, NCH, 512).transpose(3, 0, 2, 1, 4)
    return np.ascontiguousarray(a.reshape(H * S // 512, 512))


def _pack_wqk(wt_bf):
    """[H, CW] -> pretiled [(hf kb p ki), 256]: 2KB contiguous per partition row."""
    a = wt_bf.reshape(8, 4, P, 2, 256).transpose(3, 0, 2, 1, 4)
    return np.ascontiguousarray(a.reshape(H * CW // 256, 256))


def _pack_wv(wt_bf):
    """[H, CW] -> pretiled [(kb p ki), 512]: 4KB contiguous per partition row."""
    a = wt_bf.reshape(8, 4, P, 512).transpose(0, 2, 1, 3)
    return np.ascontiguousarray(a.reshape(H * CW // 512, 512))


def kernel(hidden_states, Wq, Wk, Wv, Wo, Aq, Bq, Ak, Bk, Av, Bv):
    if "nc" not in _CACHE:
        _CACHE["nc"] = _build()
    nc = _CACHE["nc"]

    x = np.ascontiguousarray(np.asarray(hidden_states, dtype=np.float32)[0])  # (S, H)
    xt_bf = np.ascontiguousarray(x.T).astype(ml_dtypes.bfloat16)
    xt_pk = _pack_x(xt_bf)

    Wq_eff = _fold(np.asarray(Wq), np.asarray(Aq), np.asarray(Bq))
    Wk_eff = _fold(np.asarray(Wk), np.asarray(Ak), np.asarray(Bk))
    Wv_eff = _fold(np.asarray(Wv), np.asarray(Av), np.asarray(Bv))
    Wo_np = np.asarray(Wo, dtype=np.float32)

    cs, tri, onesb, onesr = _host_consts()

    in_maps = []
    for c in range(NCORES):
        cols = slice(CW * c, CW * (c + 1))
        in_maps.append({
            "xt": xt_pk,
            "wq": _pack_wqk(Wq_eff[cols].T.astype(ml_dtypes.bfloat16)),
            "wk": _pack_wqk(Wk_eff[cols].T.astype(ml_dtypes.bfloat16)),
            "wv": _pack_wv(Wv_eff[cols].T.astype(ml_dtypes.bfloat16)),
            "wot": np.ascontiguousarray(Wo_np[:, cols].T).astype(ml_dtypes.bfloat16),
            "cs": cs,
            "tri": tri,
            "onesb": onesb,
            "onesr": onesr,
        })
    _CACHE["in_maps"] = in_maps

    res = bass_utils.run_bass_kernel_spmd(nc, in_maps, core_ids=list(range(NCORES)))
    acc = np.zeros((S, H), dtype=np.float32)
    for c in range(NCORES):
        acc += res.results[c]["out"].astype(np.float32)
    return acc[None]

